# revision 27
# baseline (speedup 1.0000x reference)
"""MemoryEfficientAttention on 8 TRN2 NeuronCores.

Full inputs in, full output out. Sharding: data-parallel over batch (2) x
tensor-parallel over heads (16 heads -> 4 heads/core). Each core computes
qkv projection for its heads, attention, and a partial output projection
over its 256 head-dims; the host sums the 4 partial projections per batch
(bf16 partials, f32 accumulate) and adds the bias.

Schedule (the whole point of this kernel): the Act engine's 128 exp
instructions (~1.04us each on [128,1024] psum tiles) are the throughput
floor (~135us). Everything else hides under them:
  - value path in bf16 (1 cyc/row matmuls, rel err ~7e-3), psum f32
  - S matmuls issue 2 iterations ahead of the PV accumulation so the PE
    never blocks Act (psum tag-ps double-buffered, eb triple-buffered)
  - lead-in order K -> Q(query-tile 0) -> V, psum->sbuf copies alternate
    DVE/Act; x streams in column-chunks so matmuls start after ~1/4 load
  - Q(tile 1) and the output projection of tile 0 are interleaved as
    fillers into the Act-bound attention windows; only tile 1's
    projection remains as a tail
  - softmax: exp(S*scale - 1) with the bias folded out by normalization
    (keeps bf16 exp values in a good range); denominator via a ones
    column appended to V (Z lands on psum partition 64); 1/Z broadcast
    via gpsimd, one DVE multiply into the bf16 ot tiles
"""

import numpy as np

B, N, C = 2, 2048, 1024
H, HD = 16, 64
NCORES = 8
TPG = 4              # tensor-parallel cores per batch
HPC = H // TPG       # 4 heads per core
D = HPC * HD         # 256 local head dims
KO = C // 128        # 8 contraction subtiles of the model dim
NB = N // 128        # 16 token blocks
MB = N // 128        # 16 key blocks
NT = 1024            # query-tile width in attention
NTC = N // NT
SCALE = HD ** -0.5
EBIAS = -1.0         # exp bias, cancelled exactly by the 1/Z normalize

_state = {}


def _build_nc(reps=1, phase="full", dtype="bf16", opts=None):
    import concourse.bass as bass
    import concourse.tile as tile
    import concourse.mybir as mybir
    from concourse import bacc

    opts = {**dict(pipe=True, pipe_depth=4, fill=True, eb_bufs=6,
                   ps_bufs=3, po_bufs=1,
                   alt_copy=True, po_alt=False, fill_stride=0),
            **(opts or {})}
    f32 = mybir.dt.float32
    mdt = mybir.dt.bfloat16
    Exp = mybir.ActivationFunctionType.Exp
    mult = mybir.AluOpType.mult

    nc = bacc.Bacc("TRN2", target_bir_lowering=False, debug=False,
                   num_devices=NCORES)

    xT_d = nc.dram_tensor("xT", [C, N], mdt, kind="ExternalInput")
    wqT_d = nc.dram_tensor("wqT", [C, D], mdt, kind="ExternalInput")
    wkT_d = nc.dram_tensor("wkT", [C, D], mdt, kind="ExternalInput")
    wvT_d = nc.dram_tensor("wvT", [C, D], mdt, kind="ExternalInput")
    pwT_d = nc.dram_tensor("pwT", [D, C], mdt, kind="ExternalInput")
    y_d = nc.dram_tensor("y", [N, C], mdt, kind="ExternalOutput")

    with tile.TileContext(nc) as tc:
        with (
            tc.tile_pool(name="big", bufs=1) as big,
            tc.tile_pool(name="work", bufs=2) as work,
            tc.tile_pool(name="ebp", bufs=opts["eb_bufs"]) as ebp,
            tc.tile_pool(name="outp", bufs=3) as outp,
            tc.tile_pool(name="ps_s", bufs=opts["ps_bufs"], space="PSUM") as ps_s,
            tc.tile_pool(name="ps_o", bufs=opts["po_bufs"], space="PSUM") as ps_o,
        ):
            ebias = big.tile([128, 1], f32, tag="ebias")
            ones_sb = big.tile([128, NB * HPC], f32, tag="ones_sb")
            nc.vector.memset(ebias[:], EBIAS)
            nc.vector.memset(ones_sb[:], 1.0)

            # alternate psum->sbuf copies between DVE and Act
            cop_i = [0]

            def copy(dst, src, engine=None):
                if engine is None:
                    engine = ("dve", "act")[cop_i[0] % 2] \
                        if opts["alt_copy"] else "dve"
                    cop_i[0] += 1
                if engine == "act":
                    nc.scalar.copy(dst, src)
                else:
                    nc.vector.tensor_copy(dst, src)

            def emit_body():
                # per-rep tiles, double-buffered so rep r+1's loads and qkv
                # overlap rep r's attention/tail in the hardware loop
                xt = big.tile([128, KO, N], mdt, tag="xt", bufs=2, name="xt")
                wq = big.tile([128, KO, D], mdt, tag="wq", bufs=1, name="wq")
                wk = big.tile([128, KO, D], mdt, tag="wk", bufs=1, name="wk")
                wv = big.tile([128, KO, D], mdt, tag="wv", bufs=1, name="wv")
                pw = big.tile([128, D // 128, C], mdt, tag="pw", bufs=1,
                              name="pw")
                qt = [big.tile([128, N], mdt, tag=f"qt{t}", bufs=2,
                               name=f"qt{t}") for t in range(2)]
                kt = [big.tile([128, N], mdt, tag=f"kt{t}", bufs=2,
                               name=f"kt{t}") for t in range(2)]
                vt = big.tile([128, NB, HPC * (HD + 1)], mdt, tag="vt",
                              bufs=2, name="vt")
                ot = [big.tile([128, N], mdt, tag=f"ot{t}", bufs=1,
                               name=f"ot{t}") for t in range(2)]
                vt4 = vt[:].rearrange("p nb (h c) -> p nb h c", c=HD + 1)
                nc.vector.tensor_copy(
                    vt4[:, :, :, HD:HD + 1],
                    ones_sb[:].rearrange("p (nb h) -> p nb h", nb=NB
                                         ).unsqueeze(-1))
                # ---- loads ----
                # HWDGE dispatch is serial (~0.6us/transfer): keep transfer
                # count low but column-major on x so the K matmuls start
                # after ~2 of its 8 transfers. Contiguous runs >= 512B.
                for half in range(2):
                    ksl = slice(half * 4, (half + 1) * 4)
                    nc.sync.dma_start(
                        wk[:, ksl, :],
                        wkT_d.ap()[half * 512:(half + 1) * 512, :]
                        .rearrange("(ko p) d -> p ko d", p=128))
                for j in range(8):
                    csl = slice(j * 256, (j + 1) * 256)
                    nc.sync.dma_start(
                        xt[:, :, csl],
                        xT_d.ap()[:, csl].rearrange("(ko p) n -> p ko n",
                                                    p=128))
                    if j == 1:
                        for half in range(2):
                            ksl = slice(half * 4, (half + 1) * 4)
                            nc.sync.dma_start(
                                wq[:, ksl, :],
                                wqT_d.ap()[half * 512:(half + 1) * 512, :]
                                .rearrange("(ko p) d -> p ko d", p=128))
                    if j == 3:
                        for half in range(2):
                            ksl = slice(half * 4, (half + 1) * 4)
                            nc.sync.dma_start(
                                wv[:, ksl, :],
                                wvT_d.ap()[half * 512:(half + 1) * 512, :]
                                .rearrange("(ko p) d -> p ko d", p=128))
                nc.sync.dma_start(
                    pw[:], pwT_d.ap().rearrange("(t p) e -> p t e", p=128))

                # ---- lead-in: K (all), Q(nt0), V (all) ----
                # during the lead-in the attention po pool is idle: alternate
                # psum allocs across both pools for 4-deep pipelining
                qp_i = [0]

                def qkv_psum():
                    qp_i[0] += 1
                    pool = (ps_s, ps_o)[qp_i[0] % 2]
                    tag = ("ps", "po")[qp_i[0] % 2]
                    return pool.tile([128, NT], f32, tag=tag, name="pm")

                qk_open = {}

                def qk_group(w, dst, t, ch, during_attn=False, half=None):
                    csl = slice(ch * 512, (ch + 1) * 512)
                    dsl = slice(t * 128, (t + 1) * 128)
                    if half == 1:
                        pm = qk_open.pop((t, ch))
                        kos = range(KO // 2, KO)
                    else:
                        pm = (ps_s.tile([128, NT], f32, tag="ps", name="pm")
                              if during_attn else qkv_psum())
                        kos = range(KO // 2) if half == 0 else range(KO)
                        if half == 0:
                            qk_open[(t, ch)] = pm
                    for ko in kos:
                        nc.tensor.matmul(
                            pm[:, 0:512], w[:, ko, dsl], xt[:, ko, csl],
                            start=(ko == 0), stop=(ko == KO - 1))
                    if half != 0:
                        copy(dst[t][:, csl], pm[:, 0:512],
                             engine="dve" if during_attn else None)

                def v_group(m):
                    # V for token blocks 2m, 2m+1 in one psum tile
                    pm = qkv_psum()
                    for j in range(2):
                        bsl = slice((2 * m + j) * 128, (2 * m + j + 1) * 128)
                        for ko in range(KO):
                            nc.tensor.matmul(
                                pm[:, j * D:(j + 1) * D],
                                xt[:, ko, bsl], wv[:, ko, :],
                                start=(ko == 0), stop=(ko == KO - 1))
                    copy(vt4[:, 2 * m:2 * m + 2, :, 0:HD],
                         pm[:, 0:512].rearrange("p (nb h c) -> p nb h c",
                                                nb=2, c=HD))

                for ch in range(4):
                    for t in range(2):
                        qk_group(wk, kt, t, ch)
                for ch in range(2):
                    for t in range(2):
                        qk_group(wq, qt, t, ch)
                for m in range(NB // 2):
                    v_group(m)

                # ---- attention + interleaved projection ----
                pj_open = {}

                def proj_unit(nb, during_attn, half=None):
                    bsl = slice(nb * 128, (nb + 1) * 128)
                    if half == 1:
                        py = pj_open.pop(nb)
                        ts = (1,)
                    else:
                        py = ps_s.tile([128, NT], f32, tag="ps", name="py")
                        ts = (0,) if half == 0 else (0, 1)
                        if half == 0:
                            pj_open[nb] = py
                    for t in ts:
                        for ech in range(2):
                            esl = slice(ech * 512, (ech + 1) * 512)
                            nc.tensor.matmul(
                                py[:, esl], ot[t][:, bsl], pw[:, t, esl],
                                start=(t == 0), stop=(t == 1))
                    if half == 0:
                        return
                    ybig = outp.tile([128, C], mdt, tag="ybig", name="ybig")
                    copy(ybig[:], py[:],
                         engine="dve" if during_attn else None)
                    nc.sync.dma_start(y_d.ap()[bsl, :], ybig[:])

                def emit_S(nt, h, mb):
                    t, hi = divmod(h, 2)
                    psl = slice(hi * 64, (hi + 1) * 64)
                    psb = ps_s.tile([128, NT], f32, tag="ps", name="psb")
                    msl = slice(mb * 128, (mb + 1) * 128)
                    for sc in range(2):
                        nc.tensor.matmul(
                            psb[:, sc * 512:(sc + 1) * 512],
                            kt[t][psl, msl],
                            qt[t][psl, nt * NT + sc * 512:
                                  nt * NT + (sc + 1) * 512],
                            start=True, stop=True)
                    eb = ebp.tile([128, NT], mdt, tag="eb", name="eb")
                    nc.scalar.activation(
                        out=eb[:], in_=psb[:], func=Exp,
                        scale=SCALE, bias=ebias[:])
                    return eb

                def emit_PV(po, nt, h, mb, eb):
                    for sc in range(2):
                        nc.tensor.matmul(
                            po[:, sc * 512:(sc + 1) * 512],
                            vt4[:, mb, h, :],
                            eb[:, sc * 512:(sc + 1) * 512],
                            start=(mb == 0), stop=(mb == MB - 1))

                def normalize(po, nt, h):
                    # ot[psl, qsl] = po[0:HD] * (1/Z); Z on psum partition 64.
                    # One copy frees po (single psum buf) for the next head;
                    # the normalize chain then runs off the critical path.
                    t, hi = divmod(h, 2)
                    psl = slice(hi * 64, (hi + 1) * 64)
                    qsl = slice(nt * NT, (nt + 1) * NT)
                    poc = work.tile([HD + 1, NT], f32, tag="poc", name="poc")
                    nc.vector.tensor_copy(poc[:], po[:])
                    rz = work.tile([1, NT], f32, tag="rz", name="rz")
                    nc.vector.reciprocal(rz[:], poc[HD:HD + 1, :])
                    rzb = work.tile([64, NT], f32, tag="rzb", name="rzb")
                    nc.gpsimd.partition_broadcast(rzb[:], rz[:])
                    nc.vector.tensor_tensor(
                        ot[t][psl, qsl], poc[0:HD, :], rzb[:], mult)

                # one flat software-pipelined stream over all (nt, h, mb):
                # S issues PIPE iterations ahead of the PV that consumes it,
                # across head/tile boundaries, so Act's exp stream never
                # drains. Fillers (Q of tile 1, proj of tile 0) slot into the
                # PV cursor's window.
                iters = [(nt, h, mb) for nt in range(NTC)
                         for h in range(HPC) for mb in range(MB)]
                PIPE = opts["pipe_depth"] if opts["pipe"] else 0
                if opts["fill"]:
                    # fillers spread thin across the mb slots: HW strongly
                    # prefers smooth small perturbations of the S/exp cadence
                    # over bursts (boundary batching measured +56us)
                    fills = {}
                    # phase-0 holds a psum buf open: phase-1 must follow
                    # within 2 slots (3-buf round-robin) or the S stream
                    # would reuse the open buffer and deadlock
                    for u, (t, ch) in enumerate(
                            (t, ch) for ch in range(2, 4) for t in range(2)):
                        fills[8 + 14 * u] = (lambda t=t, ch=ch:
                                             qk_group(wq, qt, t, ch, True, 0))
                        fills[10 + 14 * u] = (lambda t=t, ch=ch:
                                              qk_group(wq, qt, t, ch, True, 1))
                    for u in range(NB // 2):
                        fills[66 + 7 * u] = lambda nb=u: proj_unit(nb, True, 0)
                        fills[68 + 7 * u] = lambda nb=u: proj_unit(nb, True, 1)
                else:
                    fills = {}
                    for ch in range(2, 4):
                        for t in range(2):
                            qk_group(wq, qt, t, ch)
                ebq = []
                po_cur = [None]
                for i in range(len(iters) + PIPE):
                    if i < len(iters):
                        ebq.append(emit_S(*iters[i]))
                    j = i - PIPE
                    if j >= 0:
                        nt, h, mb = iters[j]
                        if mb == 0:
                            pool = (ps_o if (not opts["po_alt"] or h % 2 == 0)
                                    else ps_s)
                            tag = "po" if pool is ps_o else "ps"
                            po_cur[0] = pool.tile([HD + 1, NT], f32,
                                                  tag=tag, name="po")
                        if j in fills:
                            fills.pop(j)()
                        emit_PV(po_cur[0], nt, h, mb, ebq.pop(0))
                        if mb == MB - 1:
                            normalize(po_cur[0], nt, h)
                for u in range(NB // 2):
                    proj_unit(NB // 2 + u, False, None)

            if opts.get("unroll"):
                for _ in range(opts["unroll"]):
                    emit_body()
            elif reps <= 2:
                for _ in range(reps):
                    emit_body()
            else:
                # device-side hardware loop with TWO bodies per iteration:
                # a tile tag with bufs=2 then genuinely alternates buffers
                # between consecutive executions (a single body in a hw loop
                # has static addresses), letting rep r+1's loads/qkv overlap
                # rep r's attention tail across the loop boundary.
                assert reps % 2 == 0, reps
                with tc.For_i(0, reps // 2, 1):
                    emit_body()
                    emit_body()

    nc.compile()
    return nc


def _get_nc(reps=1, phase="full", dtype="bf16", opts=None):
    key = f"nc{reps}-{phase}-{dtype}-{sorted((opts or {}).items())}"
    if key not in _state:
        _state[key] = _build_nc(reps, phase, dtype, opts)
    return _state[key]


def _shard_inputs(x, qkv_w, proj_w, dtype="bf16"):
    """Per-core input maps. Core c: batch c//4, heads 4*(c%4)..4*(c%4)+3."""
    import ml_dtypes
    cast = lambda a: np.ascontiguousarray(a).astype(ml_dtypes.bfloat16)
    in_maps = []
    for c in range(NCORES):
        b, g = divmod(c, TPG)
        dsl = slice(g * D, (g + 1) * D)
        in_maps.append({
            "xT": cast(x[b].T),
            "wqT": cast(qkv_w[dsl, :].T),
            "wkT": cast(qkv_w[C:2 * C][dsl, :].T),
            "wvT": cast(qkv_w[2 * C:][dsl, :].T),
            "pwT": cast(proj_w[:, dsl].T),
        })
    return in_maps


def _make_runner(nc, donate=True):
    """Jitted 8-core SPMD runner for a built Bass module."""
    import jax
    import concourse.mybir as mybir
    from concourse import bass2jax

    bass2jax.install_neuronx_cc_hook()

    partition_name = (nc.partition_id_tensor.name
                      if nc.partition_id_tensor else None)
    in_names, out_names, out_avals, zero_shapes = [], [], [], []
    for alloc in nc.m.functions[0].allocations:
        if not isinstance(alloc, mybir.MemoryLocationSet):
            continue
        name = alloc.memorylocations[0].name
        if alloc.kind == "ExternalInput":
            if name != partition_name:
                in_names.append(name)
        elif alloc.kind == "ExternalOutput":
            shape = tuple(alloc.tensor_shape)
            dtype = mybir.dt.np(alloc.dtype)
            out_names.append(name)
            out_avals.append(jax.core.ShapedArray(shape, dtype))
            zero_shapes.append((shape, dtype))
    n_params = len(in_names)
    all_in_names = list(in_names) + list(out_names)
    if partition_name is not None:
        all_in_names.append(partition_name)
    donate_idx = tuple(range(n_params, n_params + len(out_names))) if donate \
        else ()

    def _body(*args):
        operands = list(args)
        if partition_name is not None:
            operands.append(bass2jax.partition_id_tensor())
        outs = bass2jax._bass_exec_p.bind(
            *operands,
            out_avals=tuple(out_avals),
            in_names=tuple(all_in_names),
            out_names=tuple(out_names),
            lowering_input_output_aliases=(),
            sim_require_finite=True,
            sim_require_nnan=True,
            nc=nc,
        )
        return tuple(outs)

    devices = jax.devices()[:NCORES]
    mesh = bass2jax.Mesh(np.asarray(devices), ("core",))
    spec = (bass2jax.PartitionSpec("core"),)
    sharded = jax.jit(
        bass2jax.shard_map(
            _body, mesh=mesh,
            in_specs=spec * (n_params + len(out_names)),
            out_specs=spec * len(out_names),
            check_rep=False),
        donate_argnums=donate_idx, keep_unused=True)

    meta = dict(in_names=in_names, out_names=out_names, out_avals=out_avals,
                zero_shapes=zero_shapes, mesh=mesh)
    return sharded, meta


def _get_runner():
    if "runner" in _state:
        return _state["runner"]
    nc = _get_nc(1)
    sharded, meta = _make_runner(nc, donate=True)

    def run(in_maps):
        concat_in = [
            np.concatenate([np.asarray(m[name]) for m in in_maps], axis=0)
            for name in meta["in_names"]
        ]
        concat_zeros = [
            np.zeros((NCORES * s[0], *s[1:]), dt)
            for s, dt in meta["zero_shapes"]
        ]
        out_arrs = sharded(*concat_in, *concat_zeros)
        out_avals = meta["out_avals"]
        return [
            {name: np.asarray(out_arrs[i]).reshape(
                NCORES, *out_avals[i].shape)[c]
             for i, name in enumerate(meta["out_names"])}
            for c in range(NCORES)
        ]

    _state["runner"] = run
    return run


def _combine(results, proj_b):
    """Sum the 4 tensor-parallel partial projections per batch, add bias."""
    out = np.empty((B, N, C), np.float32)
    for b in range(B):
        acc = results[b * TPG + 0]["y"].astype(np.float32)
        for g in range(1, TPG):
            acc += results[b * TPG + g]["y"].astype(np.float32)
        out[b] = acc + proj_b[None, :]
    return out


def kernel(x, qkv_w, proj_w, proj_b):
    x = np.asarray(x, np.float32)
    qkv_w = np.asarray(qkv_w, np.float32)
    proj_w = np.asarray(proj_w, np.float32)
    proj_b = np.asarray(proj_b, np.float32)
    run = _get_runner()
    results = run(_shard_inputs(x, qkv_w, proj_w))
    return _combine(results, proj_b)


def make_timing_fn(reps, in_maps, phase="full", dtype="bf16", opts=None):
    """Device-resident, non-donating executor of the reps-times kernel.

    Returns fn() that launches one execution and blocks until done. Inputs
    (and dummy zero outputs) are placed on device once, so repeated calls
    measure dispatch + on-device execution only.
    """
    import jax
    from jax.sharding import NamedSharding
    from concourse import bass2jax

    nc = _get_nc(reps, phase, dtype, opts)
    sharded, meta = _make_runner(nc, donate=False)
    shd = NamedSharding(meta["mesh"], bass2jax.PartitionSpec("core"))
    dev_in = [
        jax.device_put(
            np.concatenate([np.asarray(m[name]) for m in in_maps], axis=0),
            shd)
        for name in meta["in_names"]
    ]
    dev_zero = [
        jax.device_put(np.zeros((NCORES * s[0], *s[1:]), dt), shd)
        for s, dt in meta["zero_shapes"]
    ]

    def fn():
        outs = sharded(*dev_in, *dev_zero)
        for o in outs:
            o.block_until_ready()
        return outs

    return fn


# revision 28
# speedup vs baseline: 1.0073x; 1.0073x over previous
"""MemoryEfficientAttention on 8 TRN2 NeuronCores.

Full inputs in, full output out. Sharding: data-parallel over batch (2) x
tensor-parallel over heads (16 heads -> 4 heads/core). Each core computes
qkv projection for its heads, attention, and a partial output projection
over its 256 head-dims; the host sums the 4 partial projections per batch
(bf16 partials, f32 accumulate) and adds the bias.

Schedule (the whole point of this kernel): the Act engine's 128 exp
instructions (~1.04us each on [128,1024] psum tiles) are the throughput
floor (~135us). Everything else hides under them:
  - value path in bf16 (1 cyc/row matmuls, rel err ~7e-3), psum f32
  - S matmuls issue 2 iterations ahead of the PV accumulation so the PE
    never blocks Act (psum tag-ps double-buffered, eb triple-buffered)
  - lead-in order K -> Q(query-tile 0) -> V, psum->sbuf copies alternate
    DVE/Act; x streams in column-chunks so matmuls start after ~1/4 load
  - Q(tile 1) and the output projection of tile 0 are interleaved as
    fillers into the Act-bound attention windows; only tile 1's
    projection remains as a tail
  - softmax: exp(S*scale - 1) with the bias folded out by normalization
    (keeps bf16 exp values in a good range); denominator via a ones
    column appended to V (Z lands on psum partition 64); 1/Z broadcast
    via gpsimd, one DVE multiply into the bf16 ot tiles
"""

import numpy as np

B, N, C = 2, 2048, 1024
H, HD = 16, 64
NCORES = 8
TPG = 4              # tensor-parallel cores per batch
HPC = H // TPG       # 4 heads per core
D = HPC * HD         # 256 local head dims
KO = C // 128        # 8 contraction subtiles of the model dim
NB = N // 128        # 16 token blocks
MB = N // 128        # 16 key blocks
NT = 1024            # query-tile width in attention
NTC = N // NT
SCALE = HD ** -0.5
EBIAS = -1.0         # exp bias, cancelled exactly by the 1/Z normalize

_state = {}


def _build_nc(reps=1, phase="full", dtype="bf16", opts=None):
    import concourse.bass as bass
    import concourse.tile as tile
    import concourse.mybir as mybir
    from concourse import bacc

    opts = {**dict(pipe=True, pipe_depth=2, fill=True, eb_bufs=6,
                   ps_bufs=3, po_bufs=1,
                   alt_copy=True, po_alt=False, fill_stride=0),
            **(opts or {})}
    f32 = mybir.dt.float32
    mdt = mybir.dt.bfloat16
    Exp = mybir.ActivationFunctionType.Exp
    mult = mybir.AluOpType.mult

    nc = bacc.Bacc("TRN2", target_bir_lowering=False, debug=False,
                   num_devices=NCORES)

    xT_d = nc.dram_tensor("xT", [C, N], mdt, kind="ExternalInput")
    wqT_d = nc.dram_tensor("wqT", [C, D], mdt, kind="ExternalInput")
    wkT_d = nc.dram_tensor("wkT", [C, D], mdt, kind="ExternalInput")
    wvT_d = nc.dram_tensor("wvT", [C, D], mdt, kind="ExternalInput")
    pwT_d = nc.dram_tensor("pwT", [D, C], mdt, kind="ExternalInput")
    y_d = nc.dram_tensor("y", [N, C], mdt, kind="ExternalOutput")

    with tile.TileContext(nc) as tc:
        with (
            tc.tile_pool(name="big", bufs=1) as big,
            tc.tile_pool(name="work", bufs=2) as work,
            tc.tile_pool(name="ebp", bufs=opts["eb_bufs"]) as ebp,
            tc.tile_pool(name="outp", bufs=3) as outp,
            tc.tile_pool(name="ps_s", bufs=opts["ps_bufs"], space="PSUM") as ps_s,
            tc.tile_pool(name="ps_o", bufs=opts["po_bufs"], space="PSUM") as ps_o,
        ):
            ebias = big.tile([128, 1], f32, tag="ebias")
            ones_sb = big.tile([128, NB * HPC], f32, tag="ones_sb")
            nc.vector.memset(ebias[:], EBIAS)
            nc.vector.memset(ones_sb[:], 1.0)

            # alternate psum->sbuf copies between DVE and Act
            cop_i = [0]

            def copy(dst, src, engine=None):
                if engine is None:
                    engine = ("dve", "act")[cop_i[0] % 2] \
                        if opts["alt_copy"] else "dve"
                    cop_i[0] += 1
                if engine == "act":
                    nc.scalar.copy(dst, src)
                else:
                    nc.vector.tensor_copy(dst, src)

            def emit_body():
                # per-rep tiles, double-buffered so rep r+1's loads and qkv
                # overlap rep r's attention/tail in the hardware loop
                xt = big.tile([128, KO, N], mdt, tag="xt", bufs=2, name="xt")
                wq = big.tile([128, KO, D], mdt, tag="wq", bufs=1, name="wq")
                wk = big.tile([128, KO, D], mdt, tag="wk", bufs=1, name="wk")
                wv = big.tile([128, KO, D], mdt, tag="wv", bufs=1, name="wv")
                pw = big.tile([128, D // 128, C], mdt, tag="pw", bufs=1,
                              name="pw")
                qt = [big.tile([128, N], mdt, tag=f"qt{t}", bufs=2,
                               name=f"qt{t}") for t in range(2)]
                kt = [big.tile([128, N], mdt, tag=f"kt{t}", bufs=2,
                               name=f"kt{t}") for t in range(2)]
                vt = big.tile([128, NB, HPC * (HD + 1)], mdt, tag="vt",
                              bufs=2, name="vt")
                ot = [big.tile([128, N], mdt, tag=f"ot{t}", bufs=1,
                               name=f"ot{t}") for t in range(2)]
                vt4 = vt[:].rearrange("p nb (h c) -> p nb h c", c=HD + 1)
                nc.vector.tensor_copy(
                    vt4[:, :, :, HD:HD + 1],
                    ones_sb[:].rearrange("p (nb h) -> p nb h", nb=NB
                                         ).unsqueeze(-1))
                # ---- loads ----
                # HWDGE dispatch is serial (~0.6us/transfer): keep transfer
                # count low but column-major on x so the K matmuls start
                # after ~2 of its 8 transfers. Contiguous runs >= 512B.
                for half in range(2):
                    ksl = slice(half * 4, (half + 1) * 4)
                    nc.sync.dma_start(
                        wk[:, ksl, :],
                        wkT_d.ap()[half * 512:(half + 1) * 512, :]
                        .rearrange("(ko p) d -> p ko d", p=128))
                for j in range(8):
                    csl = slice(j * 256, (j + 1) * 256)
                    nc.sync.dma_start(
                        xt[:, :, csl],
                        xT_d.ap()[:, csl].rearrange("(ko p) n -> p ko n",
                                                    p=128))
                    if j == 1:
                        for half in range(2):
                            ksl = slice(half * 4, (half + 1) * 4)
                            nc.sync.dma_start(
                                wq[:, ksl, :],
                                wqT_d.ap()[half * 512:(half + 1) * 512, :]
                                .rearrange("(ko p) d -> p ko d", p=128))
                    if j == 3:
                        for half in range(2):
                            ksl = slice(half * 4, (half + 1) * 4)
                            nc.sync.dma_start(
                                wv[:, ksl, :],
                                wvT_d.ap()[half * 512:(half + 1) * 512, :]
                                .rearrange("(ko p) d -> p ko d", p=128))
                nc.sync.dma_start(
                    pw[:], pwT_d.ap().rearrange("(t p) e -> p t e", p=128))

                # ---- lead-in: K (all), Q(nt0), V (all) ----
                # during the lead-in the attention po pool is idle: alternate
                # psum allocs across both pools for 4-deep pipelining
                qp_i = [0]

                def qkv_psum():
                    qp_i[0] += 1
                    pool = (ps_s, ps_o)[qp_i[0] % 2]
                    tag = ("ps", "po")[qp_i[0] % 2]
                    return pool.tile([128, NT], f32, tag=tag, name="pm")

                qk_open = {}

                def qk_group(w, dst, t, ch, during_attn=False, half=None):
                    csl = slice(ch * 512, (ch + 1) * 512)
                    dsl = slice(t * 128, (t + 1) * 128)
                    if half == 1:
                        pm = qk_open.pop((t, ch))
                        kos = range(KO // 2, KO)
                    else:
                        pm = (ps_s.tile([128, NT], f32, tag="ps", name="pm")
                              if during_attn else qkv_psum())
                        kos = range(KO // 2) if half == 0 else range(KO)
                        if half == 0:
                            qk_open[(t, ch)] = pm
                    for ko in kos:
                        nc.tensor.matmul(
                            pm[:, 0:512], w[:, ko, dsl], xt[:, ko, csl],
                            start=(ko == 0), stop=(ko == KO - 1))
                    if half != 0:
                        copy(dst[t][:, csl], pm[:, 0:512],
                             engine="dve" if during_attn else None)

                def v_group(m):
                    # V for token blocks 2m, 2m+1 in one psum tile
                    pm = qkv_psum()
                    for j in range(2):
                        bsl = slice((2 * m + j) * 128, (2 * m + j + 1) * 128)
                        for ko in range(KO):
                            nc.tensor.matmul(
                                pm[:, j * D:(j + 1) * D],
                                xt[:, ko, bsl], wv[:, ko, :],
                                start=(ko == 0), stop=(ko == KO - 1))
                    copy(vt4[:, 2 * m:2 * m + 2, :, 0:HD],
                         pm[:, 0:512].rearrange("p (nb h c) -> p nb h c",
                                                nb=2, c=HD))

                for ch in range(4):
                    for t in range(2):
                        qk_group(wk, kt, t, ch)
                for ch in range(2):
                    for t in range(2):
                        qk_group(wq, qt, t, ch)
                for m in range(NB // 2):
                    v_group(m)

                # ---- attention + interleaved projection ----
                pj_open = {}

                def proj_unit(nb, during_attn, half=None):
                    bsl = slice(nb * 128, (nb + 1) * 128)
                    if half == 1:
                        py = pj_open.pop(nb)
                        ts = (1,)
                    else:
                        py = ps_s.tile([128, NT], f32, tag="ps", name="py")
                        ts = (0,) if half == 0 else (0, 1)
                        if half == 0:
                            pj_open[nb] = py
                    for t in ts:
                        for ech in range(2):
                            esl = slice(ech * 512, (ech + 1) * 512)
                            nc.tensor.matmul(
                                py[:, esl], ot[t][:, bsl], pw[:, t, esl],
                                start=(t == 0), stop=(t == 1))
                    if half == 0:
                        return
                    ybig = outp.tile([128, C], mdt, tag="ybig", name="ybig")
                    copy(ybig[:], py[:],
                         engine="dve" if during_attn else None)
                    nc.sync.dma_start(y_d.ap()[bsl, :], ybig[:])

                def emit_S(nt, h, mb):
                    t, hi = divmod(h, 2)
                    psl = slice(hi * 64, (hi + 1) * 64)
                    psb = ps_s.tile([128, NT], f32, tag="ps", name="psb")
                    msl = slice(mb * 128, (mb + 1) * 128)
                    for sc in range(2):
                        nc.tensor.matmul(
                            psb[:, sc * 512:(sc + 1) * 512],
                            kt[t][psl, msl],
                            qt[t][psl, nt * NT + sc * 512:
                                  nt * NT + (sc + 1) * 512],
                            start=True, stop=True)
                    eb = ebp.tile([128, NT], mdt, tag="eb", name="eb")
                    nc.scalar.activation(
                        out=eb[:], in_=psb[:], func=Exp,
                        scale=SCALE, bias=ebias[:])
                    return eb

                def emit_PV(po, nt, h, mb, eb):
                    for sc in range(2):
                        nc.tensor.matmul(
                            po[:, sc * 512:(sc + 1) * 512],
                            vt4[:, mb, h, :],
                            eb[:, sc * 512:(sc + 1) * 512],
                            start=(mb == 0), stop=(mb == MB - 1))

                def normalize(po, nt, h):
                    # ot[psl, qsl] = po[0:HD] * (1/Z); Z on psum partition 64.
                    # One copy frees po (single psum buf) for the next head;
                    # the normalize chain then runs off the critical path.
                    t, hi = divmod(h, 2)
                    psl = slice(hi * 64, (hi + 1) * 64)
                    qsl = slice(nt * NT, (nt + 1) * NT)
                    poc = work.tile([HD + 1, NT], f32, tag="poc", name="poc")
                    nc.vector.tensor_copy(poc[:], po[:])
                    rz = work.tile([1, NT], f32, tag="rz", name="rz")
                    nc.vector.reciprocal(rz[:], poc[HD:HD + 1, :])
                    rzb = work.tile([64, NT], f32, tag="rzb", name="rzb")
                    nc.gpsimd.partition_broadcast(rzb[:], rz[:])
                    nc.vector.tensor_tensor(
                        ot[t][psl, qsl], poc[0:HD, :], rzb[:], mult)

                # one flat software-pipelined stream over all (nt, h, mb):
                # S issues PIPE iterations ahead of the PV that consumes it,
                # across head/tile boundaries, so Act's exp stream never
                # drains. Fillers (Q of tile 1, proj of tile 0) slot into the
                # PV cursor's window.
                iters = [(nt, h, mb) for nt in range(NTC)
                         for h in range(HPC) for mb in range(MB)]
                PIPE = opts["pipe_depth"] if opts["pipe"] else 0
                if opts["fill"]:
                    # fillers spread thin across the mb slots: HW strongly
                    # prefers smooth small perturbations of the S/exp cadence
                    # over bursts (boundary batching measured +56us)
                    fills = {}
                    # phase-0 holds a psum buf open: phase-1 must follow
                    # within 2 slots (3-buf round-robin) or the S stream
                    # would reuse the open buffer and deadlock
                    for u, (t, ch) in enumerate(
                            (t, ch) for ch in range(2, 4) for t in range(2)):
                        fills[8 + 14 * u] = (lambda t=t, ch=ch:
                                             qk_group(wq, qt, t, ch, True, 0))
                        fills[10 + 14 * u] = (lambda t=t, ch=ch:
                                              qk_group(wq, qt, t, ch, True, 1))
                    for u in range(NB // 2):
                        fills[66 + 7 * u] = lambda nb=u: proj_unit(nb, True, 0)
                        fills[68 + 7 * u] = lambda nb=u: proj_unit(nb, True, 1)
                else:
                    fills = {}
                    for ch in range(2, 4):
                        for t in range(2):
                            qk_group(wq, qt, t, ch)
                ebq = []
                po_cur = [None]
                for i in range(len(iters) + PIPE):
                    if i < len(iters):
                        ebq.append(emit_S(*iters[i]))
                    j = i - PIPE
                    if j >= 0:
                        nt, h, mb = iters[j]
                        if mb == 0:
                            pool = (ps_o if (not opts["po_alt"] or h % 2 == 0)
                                    else ps_s)
                            tag = "po" if pool is ps_o else "ps"
                            po_cur[0] = pool.tile([HD + 1, NT], f32,
                                                  tag=tag, name="po")
                        if j in fills:
                            fills.pop(j)()
                        emit_PV(po_cur[0], nt, h, mb, ebq.pop(0))
                        if mb == MB - 1:
                            normalize(po_cur[0], nt, h)
                for u in range(NB // 2):
                    proj_unit(NB // 2 + u, False, None)

            if opts.get("unroll"):
                for _ in range(opts["unroll"]):
                    emit_body()
            elif reps <= 2:
                for _ in range(reps):
                    emit_body()
            else:
                # device-side hardware loop with TWO bodies per iteration:
                # a tile tag with bufs=2 then genuinely alternates buffers
                # between consecutive executions (a single body in a hw loop
                # has static addresses), letting rep r+1's loads/qkv overlap
                # rep r's attention tail across the loop boundary.
                assert reps % 2 == 0, reps
                with tc.For_i(0, reps // 2, 1):
                    emit_body()
                    emit_body()

    nc.compile()
    return nc


def _get_nc(reps=1, phase="full", dtype="bf16", opts=None):
    key = f"nc{reps}-{phase}-{dtype}-{sorted((opts or {}).items())}"
    if key not in _state:
        _state[key] = _build_nc(reps, phase, dtype, opts)
    return _state[key]


def _shard_inputs(x, qkv_w, proj_w, dtype="bf16"):
    """Per-core input maps. Core c: batch c//4, heads 4*(c%4)..4*(c%4)+3."""
    import ml_dtypes
    cast = lambda a: np.ascontiguousarray(a).astype(ml_dtypes.bfloat16)
    in_maps = []
    for c in range(NCORES):
        b, g = divmod(c, TPG)
        dsl = slice(g * D, (g + 1) * D)
        in_maps.append({
            "xT": cast(x[b].T),
            "wqT": cast(qkv_w[dsl, :].T),
            "wkT": cast(qkv_w[C:2 * C][dsl, :].T),
            "wvT": cast(qkv_w[2 * C:][dsl, :].T),
            "pwT": cast(proj_w[:, dsl].T),
        })
    return in_maps


def _make_runner(nc, donate=True):
    """Jitted 8-core SPMD runner for a built Bass module."""
    import jax
    import concourse.mybir as mybir
    from concourse import bass2jax

    bass2jax.install_neuronx_cc_hook()

    partition_name = (nc.partition_id_tensor.name
                      if nc.partition_id_tensor else None)
    in_names, out_names, out_avals, zero_shapes = [], [], [], []
    for alloc in nc.m.functions[0].allocations:
        if not isinstance(alloc, mybir.MemoryLocationSet):
            continue
        name = alloc.memorylocations[0].name
        if alloc.kind == "ExternalInput":
            if name != partition_name:
                in_names.append(name)
        elif alloc.kind == "ExternalOutput":
            shape = tuple(alloc.tensor_shape)
            dtype = mybir.dt.np(alloc.dtype)
            out_names.append(name)
            out_avals.append(jax.core.ShapedArray(shape, dtype))
            zero_shapes.append((shape, dtype))
    n_params = len(in_names)
    all_in_names = list(in_names) + list(out_names)
    if partition_name is not None:
        all_in_names.append(partition_name)
    donate_idx = tuple(range(n_params, n_params + len(out_names))) if donate \
        else ()

    def _body(*args):
        operands = list(args)
        if partition_name is not None:
            operands.append(bass2jax.partition_id_tensor())
        outs = bass2jax._bass_exec_p.bind(
            *operands,
            out_avals=tuple(out_avals),
            in_names=tuple(all_in_names),
            out_names=tuple(out_names),
            lowering_input_output_aliases=(),
            sim_require_finite=True,
            sim_require_nnan=True,
            nc=nc,
        )
        return tuple(outs)

    devices = jax.devices()[:NCORES]
    mesh = bass2jax.Mesh(np.asarray(devices), ("core",))
    spec = (bass2jax.PartitionSpec("core"),)
    sharded = jax.jit(
        bass2jax.shard_map(
            _body, mesh=mesh,
            in_specs=spec * (n_params + len(out_names)),
            out_specs=spec * len(out_names),
            check_rep=False),
        donate_argnums=donate_idx, keep_unused=True)

    meta = dict(in_names=in_names, out_names=out_names, out_avals=out_avals,
                zero_shapes=zero_shapes, mesh=mesh)
    return sharded, meta


def _get_runner():
    if "runner" in _state:
        return _state["runner"]
    nc = _get_nc(1)
    sharded, meta = _make_runner(nc, donate=True)

    def run(in_maps):
        concat_in = [
            np.concatenate([np.asarray(m[name]) for m in in_maps], axis=0)
            for name in meta["in_names"]
        ]
        concat_zeros = [
            np.zeros((NCORES * s[0], *s[1:]), dt)
            for s, dt in meta["zero_shapes"]
        ]
        out_arrs = sharded(*concat_in, *concat_zeros)
        out_avals = meta["out_avals"]
        return [
            {name: np.asarray(out_arrs[i]).reshape(
                NCORES, *out_avals[i].shape)[c]
             for i, name in enumerate(meta["out_names"])}
            for c in range(NCORES)
        ]

    _state["runner"] = run
    return run


def _combine(results, proj_b):
    """Sum the 4 tensor-parallel partial projections per batch, add bias."""
    out = np.empty((B, N, C), np.float32)
    for b in range(B):
        acc = results[b * TPG + 0]["y"].astype(np.float32)
        for g in range(1, TPG):
            acc += results[b * TPG + g]["y"].astype(np.float32)
        out[b] = acc + proj_b[None, :]
    return out


def kernel(x, qkv_w, proj_w, proj_b):
    x = np.asarray(x, np.float32)
    qkv_w = np.asarray(qkv_w, np.float32)
    proj_w = np.asarray(proj_w, np.float32)
    proj_b = np.asarray(proj_b, np.float32)
    run = _get_runner()
    results = run(_shard_inputs(x, qkv_w, proj_w))
    return _combine(results, proj_b)


def make_timing_fn(reps, in_maps, phase="full", dtype="bf16", opts=None):
    """Device-resident, non-donating executor of the reps-times kernel.

    Returns fn() that launches one execution and blocks until done. Inputs
    (and dummy zero outputs) are placed on device once, so repeated calls
    measure dispatch + on-device execution only.
    """
    import jax
    from jax.sharding import NamedSharding
    from concourse import bass2jax

    nc = _get_nc(reps, phase, dtype, opts)
    sharded, meta = _make_runner(nc, donate=False)
    shd = NamedSharding(meta["mesh"], bass2jax.PartitionSpec("core"))
    dev_in = [
        jax.device_put(
            np.concatenate([np.asarray(m[name]) for m in in_maps], axis=0),
            shd)
        for name in meta["in_names"]
    ]
    dev_zero = [
        jax.device_put(np.zeros((NCORES * s[0], *s[1:]), dt), shd)
        for s, dt in meta["zero_shapes"]
    ]

    def fn():
        outs = sharded(*dev_in, *dev_zero)
        for o in outs:
            o.block_until_ready()
        return outs

    return fn
